# revision 16
# baseline (speedup 1.0000x reference)
"""Depthwise causal Conv1D (tw=4) on 8 Trainium2 NeuronCores.

Problem (hardcoded): x [8, 4096, 2048] f32, cache [8, 3, 2048] f32,
w [4, 2048] f32, b [2048] f32 -> (y [8, 4096, 2048], new_cache [8, 3, 2048]).

    y[n, t, c] = b[c] + sum_k x_cat[n, t+k, c] * w[k, c],  x_cat = [cache; x]
    new_cache  = x_cat[:, -3:, :]  (= x[:, -3:, :])

Sharding: data-parallel over batch, one batch element per core.

Device plan: host pre-transposes each batch element to channel-major
[W, T+3] (cache prepended) so every DMA is contiguous per partition, and
pre-rounds x to fp32r (RNE to 11 explicit mantissa bits) so the PE can run
full-rate fp32r matmuls. Per (channel-block, 512-time tile) the conv is 4
accumulating PE matmuls with diag(w_k) stationary operands (w_k rounded to
fp32r on-device during the diag build) against time-shifted slices of the
x tile; the PSUM result is evacuated with a fused per-partition bias add.
Operand products of two 12-bit-significand values are exact in fp32, and
PSUM accumulates in fp32, so the only error vs the fp32 reference is the
2^-13 RNE operand rounding (measured 1.9e-4 scale-relative absmax on the
reference inputs; verified bit-identical to a numpy emulation of the
rounding).

Measured: ~184-188 us HW exec (8 cores concurrent), DMA-bound at ~96%
busy / ~382 GB/s per core vs the 67.2 MB/core roofline of ~176 us.
"""

import numpy as np

import concourse.bacc as bacc
import concourse.mybir as mybir
from concourse import bass_utils, masks
from concourse.tile import TileContext

F32 = mybir.dt.float32
F32R = mybir.dt.float32r
AF = mybir.ActivationFunctionType

TW = 4
B = 8
T = 4096
W = 2048
P = 128
NCB = W // P        # 16 channel blocks per core
TT = 2048           # time tile (DMA granularity)
NTT = T // TT       # 2
SUB = 512           # matmul / PSUM sub-tile
NSUB = TT // SUB    # 4

_NC_CACHE = []


def _round_f32r(a):
    """RNE-round fp32 array to fp32r (drop low 12 mantissa bits)."""
    u = a.view(np.uint32)
    r = (u.astype(np.uint64) + 0x7FF + ((u >> 12) & 1)) & 0xFFFFF000
    return r.astype(np.uint32).view(np.float32)


def _build():
    nc = bacc.Bacc("TRN2", target_bir_lowering=False, debug=False)
    # channel-major per-core input, cache prepended along time, fp32r-rounded
    xt_d = nc.dram_tensor("xt", (W, T + TW - 1), F32R, kind="ExternalInput")
    # w/b pre-arranged on host to the exact SBUF layout (partition-major),
    # so their DMAs are one contiguous run per partition
    w_d = nc.dram_tensor("wp", (P, NCB * TW), F32, kind="ExternalInput")
    b_d = nc.dram_tensor("bp", (P, NCB), F32, kind="ExternalInput")
    y_d = nc.dram_tensor("yt", (W, T), F32, kind="ExternalOutput")

    with TileContext(nc) as tc:
        with (
            tc.tile_pool(name="const", bufs=1) as cpool,
            tc.tile_pool(name="work", bufs=8) as wpool,
            tc.tile_pool(name="yout", bufs=2) as ypool,
            tc.tile_pool(name="psum", bufs=8, space="PSUM") as ppool,
        ):
            ident = cpool.tile([P, P], F32)
            masks.make_identity(nc, ident[:])
            # consts on the scalar HWDGE ring so x loads own the sync ring
            w_sb = cpool.tile([P, NCB, TW], F32)
            nc.scalar.dma_start(
                out=w_sb[:], in_=w_d[:].rearrange("p (cb k) -> p cb k", k=TW)
            )
            b_sb = cpool.tile([P, NCB], F32)
            nc.scalar.dma_start(out=b_sb[:], in_=b_d[:])
            # all diag(w_k) stationaries upfront: [P, cb, k, P] fp32r
            diags = cpool.tile([P, NCB, TW, P], F32R)
            for cb in range(NCB):
                for k in range(TW):
                    nc.vector.tensor_scalar_mul(
                        diags[:, cb, k, :], ident[:], w_sb[:, cb, k : k + 1]
                    )

            for cb in range(NCB):
                c0 = cb * P
                diag = diags[:, cb]
                y_sb = ypool.tile([P, T], F32, tag="y")
                for tt in range(NTT):
                    t0 = tt * TT
                    x_sb = wpool.tile([P, TT + TW - 1], F32R, tag="x")
                    nc.sync.dma_start(
                        out=x_sb[:], in_=xt_d[c0 : c0 + P, t0 : t0 + TT + TW - 1]
                    )
                    for s in range(NSUB):
                        s0 = s * SUB
                        ps = ppool.tile([P, SUB], F32, tag="ps")
                        for k in range(TW):
                            nc.tensor.matmul(
                                ps[:],
                                diag[:, k, :],
                                x_sb[:, s0 + k : s0 + k + SUB],
                                start=(k == 0),
                                stop=(k == TW - 1),
                            )
                        # PSUM -> SBUF with fused per-partition bias add
                        nc.scalar.activation(
                            y_sb[:, t0 + s0 : t0 + s0 + SUB],
                            ps[:],
                            AF.Identity,
                            bias=b_sb[:, cb : cb + 1],
                            scale=1.0,
                        )
                nc.scalar.dma_start(out=y_d[c0 : c0 + P, :], in_=y_sb[:])
    nc.compile()
    return nc


def _get_nc():
    if not _NC_CACHE:
        _NC_CACHE.append(_build())
    return _NC_CACHE[0]


def kernel(x, cache, w, b):
    x = np.asarray(x, dtype=np.float32)
    cache = np.asarray(cache, dtype=np.float32)
    w = np.asarray(w, dtype=np.float32)
    b = np.asarray(b, dtype=np.float32)

    nc = _get_nc()

    # host-side staging: per-core channel-major [W, T+3], cache prepended,
    # pre-rounded to fp32r; w/b in partition-major SBUF layout
    x_cat = np.concatenate([cache, x], axis=1)          # [B, T+3, W]
    # wp[p, cb*TW+k] = w[k, cb*P+p];  bp[p, cb] = b[cb*P+p]
    wp = np.ascontiguousarray(
        w.reshape(TW, NCB, P).transpose(2, 1, 0).reshape(P, NCB * TW)
    )
    bp = np.ascontiguousarray(b.reshape(NCB, P).T)
    in_maps = []
    for n in range(B):
        in_maps.append(
            {
                "xt": _round_f32r(np.ascontiguousarray(x_cat[n].T)),
                "wp": wp,
                "bp": bp,
            }
        )

    res = bass_utils.run_bass_kernel_spmd(nc, in_maps, core_ids=list(range(B)))
    y = np.stack([res.results[n]["yt"].T for n in range(B)], axis=0)
    y = np.ascontiguousarray(y, dtype=np.float32)

    new_cache = np.ascontiguousarray(x_cat[:, 1 - TW :, :], dtype=np.float32)
    return y, new_cache


# revision 17
# speedup vs baseline: 1.0586x; 1.0586x over previous
"""Depthwise causal Conv1D (tw=4) on 8 Trainium2 NeuronCores.

Problem (hardcoded): x [8, 4096, 2048] f32, cache [8, 3, 2048] f32,
w [4, 2048] f32, b [2048] f32 -> (y [8, 4096, 2048], new_cache [8, 3, 2048]).

    y[n, t, c] = b[c] + sum_k x_cat[n, t+k, c] * w[k, c],  x_cat = [cache; x]
    new_cache  = x_cat[:, -3:, :]  (= x[:, -3:, :])

Sharding: data-parallel over batch, one batch element per core.

Device plan: host pre-transposes each batch element to channel-major
[W, T+3] (cache prepended) so every DMA is contiguous per partition, and
pre-rounds x to fp32r (RNE to 11 explicit mantissa bits) so the PE can run
full-rate fp32r matmuls. Per (channel-block, 512-time tile) the conv is 4
accumulating PE matmuls with diag(w_k) stationary operands (w_k rounded to
fp32r on-device during the diag build) against time-shifted slices of the
x tile; the PSUM result is evacuated with a fused per-partition bias add.
Operand products of two 12-bit-significand values are exact in fp32, and
PSUM accumulates in fp32, so the only error vs the fp32 reference is the
2^-13 RNE operand rounding (measured 1.9e-4 scale-relative absmax on the
reference inputs; verified bit-identical to a numpy emulation of the
rounding).

Measured: ~184-188 us HW exec (8 cores concurrent), DMA-bound at ~96%
busy / ~382 GB/s per core vs the 67.2 MB/core roofline of ~176 us.
"""

import numpy as np

import concourse.bacc as bacc
import concourse.mybir as mybir
from concourse import bass_utils, masks
from concourse.tile import TileContext

F32 = mybir.dt.float32
F32R = mybir.dt.float32r
AF = mybir.ActivationFunctionType

TW = 4
B = 8
T = 4096
W = 2048
P = 128
NCB = W // P        # 16 channel blocks per core
TT = 2048           # time tile (DMA granularity)
NTT = T // TT       # 2
SUB = 512           # matmul / PSUM sub-tile
NSUB = TT // SUB    # 4

_NC_CACHE = []


def _round_f32r(a):
    """RNE-round fp32 array to fp32r (drop low 12 mantissa bits)."""
    u = a.view(np.uint32)
    r = (u.astype(np.uint64) + 0x7FF + ((u >> 12) & 1)) & 0xFFFFF000
    return r.astype(np.uint32).view(np.float32)


def _build():
    nc = bacc.Bacc("TRN2", target_bir_lowering=False, debug=False)
    # channel-major per-core input, cache prepended along time, fp32r-rounded
    xt_d = nc.dram_tensor("xt", (W, T + TW - 1), F32R, kind="ExternalInput")
    # w/b pre-arranged on host to the exact SBUF layout (partition-major),
    # so their DMAs are one contiguous run per partition
    w_d = nc.dram_tensor("wp", (P, NCB * TW), F32, kind="ExternalInput")
    b_d = nc.dram_tensor("bp", (P, NCB), F32, kind="ExternalInput")
    y_d = nc.dram_tensor("yt", (W, T), F32, kind="ExternalOutput")

    with TileContext(nc) as tc:
        with (
            tc.tile_pool(name="const", bufs=1) as cpool,
            tc.tile_pool(name="work", bufs=8) as wpool,
            tc.tile_pool(name="yout", bufs=4) as ypool,
            tc.tile_pool(name="psum", bufs=8, space="PSUM") as ppool,
        ):
            ident = cpool.tile([P, P], F32)
            masks.make_identity(nc, ident[:])
            # consts on the scalar HWDGE ring so x loads own the sync ring
            w_sb = cpool.tile([P, NCB, TW], F32)
            nc.scalar.dma_start(
                out=w_sb[:], in_=w_d[:].rearrange("p (cb k) -> p cb k", k=TW)
            )
            b_sb = cpool.tile([P, NCB], F32)
            nc.scalar.dma_start(out=b_sb[:], in_=b_d[:])
            # all diag(w_k) stationaries upfront: [P, cb, k, P] fp32r
            diags = cpool.tile([P, NCB, TW, P], F32R)
            for cb in range(NCB):
                for k in range(TW):
                    nc.vector.tensor_scalar_mul(
                        diags[:, cb, k, :], ident[:], w_sb[:, cb, k : k + 1]
                    )

            for cb in range(NCB):
                c0 = cb * P
                diag = diags[:, cb]
                for tt in range(NTT):
                    t0 = tt * TT
                    x_sb = wpool.tile([P, TT + TW - 1], F32R, tag="x")
                    nc.sync.dma_start(
                        out=x_sb[:], in_=xt_d[c0 : c0 + P, t0 : t0 + TT + TW - 1]
                    )
                    y_sb = ypool.tile([P, TT], F32, tag="y")
                    for s in range(NSUB):
                        s0 = s * SUB
                        ps = ppool.tile([P, SUB], F32, tag="ps")
                        for k in range(TW):
                            nc.tensor.matmul(
                                ps[:],
                                diag[:, k, :],
                                x_sb[:, s0 + k : s0 + k + SUB],
                                start=(k == 0),
                                stop=(k == TW - 1),
                            )
                        # PSUM -> SBUF with fused per-partition bias add
                        nc.scalar.activation(
                            y_sb[:, s0 : s0 + SUB],
                            ps[:],
                            AF.Identity,
                            bias=b_sb[:, cb : cb + 1],
                            scale=1.0,
                        )
                    nc.scalar.dma_start(
                        out=y_d[c0 : c0 + P, t0 : t0 + TT], in_=y_sb[:]
                    )
    nc.compile()
    return nc


def _get_nc():
    if not _NC_CACHE:
        _NC_CACHE.append(_build())
    return _NC_CACHE[0]


def kernel(x, cache, w, b):
    x = np.asarray(x, dtype=np.float32)
    cache = np.asarray(cache, dtype=np.float32)
    w = np.asarray(w, dtype=np.float32)
    b = np.asarray(b, dtype=np.float32)

    nc = _get_nc()

    # host-side staging: per-core channel-major [W, T+3], cache prepended,
    # pre-rounded to fp32r; w/b in partition-major SBUF layout
    x_cat = np.concatenate([cache, x], axis=1)          # [B, T+3, W]
    # wp[p, cb*TW+k] = w[k, cb*P+p];  bp[p, cb] = b[cb*P+p]
    wp = np.ascontiguousarray(
        w.reshape(TW, NCB, P).transpose(2, 1, 0).reshape(P, NCB * TW)
    )
    bp = np.ascontiguousarray(b.reshape(NCB, P).T)
    in_maps = []
    for n in range(B):
        in_maps.append(
            {
                "xt": _round_f32r(np.ascontiguousarray(x_cat[n].T)),
                "wp": wp,
                "bp": bp,
            }
        )

    res = bass_utils.run_bass_kernel_spmd(nc, in_maps, core_ids=list(range(B)))
    y = np.stack([res.results[n]["yt"].T for n in range(B)], axis=0)
    y = np.ascontiguousarray(y, dtype=np.float32)

    new_cache = np.ascontiguousarray(x_cat[:, 1 - TW :, :], dtype=np.float32)
    return y, new_cache
